# revision 5
# baseline (speedup 1.0000x reference)
"""Cumulative (causal) LayerNorm Trainium2 Bass kernel.

Problem: inputs [B=8, K=8000, H=512] f32, gamma/beta [1, 512].
At step k, normalize frame k by mean/var computed over the prefix
inputs[:, :k+1, :] (time and feature dims), then scale/shift by gamma/beta.

Sharding: data-parallel over batch B across the 8 NeuronCores (one sample
per core), gamma/beta/constants replicated. No cross-core communication.

Per-core layout (segmented): K = NSEG * P * FS frames; frame
    k = s*(P*FS) + p*FS + f      (s = segment, p = partition, f = tile-in-seg)
Global k-tile index t = s*FS + f in [0, 64).

fp16 I/O: x and out cross HBM as fp16 (halves DMA bytes; the 2e-2 rel-err
budget has ~7x margin over fp16 quantization, verified on host). All stats
accumulate in f32 on-chip.

Per segment:
  stats:  ACT Square+accum -> per-frame sumsqs (f32); DVE tensor_scalar
          copy+accum (4x packed mode on fp16) -> per-frame sums (f32).
  scan:   native tensor_tensor_scan along each partition's FS frames, then
          one PE matmul pair: strictly-upper-triangular ustrict @ seg-totals
          (exclusive cross-partition carry) accumulated with row124-ones @
          previous segment's final cum column (scalar carry broadcast).
  apply:  one fused custom-DVE affine_mul_reduce per frame:
          out = (x*invstd + (-mean*invstd)) * gamma, single rounding to fp16.
  dma:    input chunks on the SP HW-DGE ring, output chunks on SWDGE - two
          DMA paths that stream concurrently.
Segment s+1's loads overlap segment s's stores -> no global barrier.
"""

import numpy as np

import concourse.bass as bass
import concourse.tile as tile
from concourse import bacc, mybir
from concourse import bass_utils

B, K, H = 8, 8000, 512
P = 125           # partitions
NSEG = 8          # segments per sample
FS = 8            # k-tiles per segment  (K = NSEG * P * FS)
F = NSEG * FS     # 64 k-tiles total
EPS = 1e-8
N_CORES = 8

FP32 = mybir.dt.float32
FP16 = mybir.dt.float16


def _global_k(p, t, fs=FS):
    # frame index of (partition p, global tile t)
    s, f = t // fs, t % fs
    return s * (P * fs) + p * fs + f


def _make_consts(nseg: int = NSEG):
    # invc2[:, 0:F] = -1/count, invc2[:, F:2F] = +1/count, count = (k+1)*H
    pp, tt = np.meshgrid(np.arange(P), np.arange(F), indexing="ij")
    k = _global_k(pp, tt, F // nseg).astype(np.float64)
    inv_cnt = 1.0 / ((k + 1) * H)
    invc2 = np.concatenate([-inv_cnt, inv_cnt], axis=1).astype(np.float32)
    # ustrict[q, i] = 1 if q < i  (exclusive prefix over partitions via matmul)
    ustrict = np.triu(np.ones((P, P), dtype=np.float32), k=1)
    # row124[q, i] = 1 if q == P-1: broadcasts the previous segment's final
    # cum value (held by partition P-1) to every partition via matmul.
    row124 = np.zeros((P, P), dtype=np.float32)
    row124[P - 1, :] = 1.0
    return invc2, ustrict, row124


def _build_kernel(with_beta: bool, reps: int = 1,
                  rep_barrier: bool = False, nseg: int = NSEG,
                  apply_mode: str = "amr",
                  sum_mode: str = "dve_ts",
                  sum_act_frames: int = 0) -> bass.Bass:
    # reps>1 repeats the whole pipeline (timing harness only): wall-clock
    # difference between reps=R and reps=1 isolates per-iteration HW time
    # from PJRT/axon dispatch overhead. rep_barrier approximates single-shot
    # latency by separating reps with an all-engine barrier.
    #
    # apply_mode: "amr" = fused affine_mul_reduce per frame (1x custom DVE);
    #             "ts_tt" = tensor_scalar affine (4x) + tensor_mul gamma (2x).
    # sum_act_frames: how many of each segment's FS frame-sums go to ACT
    #             (Identity+accum) instead of DVE tensor_scalar+accum.
    #
    # Bacc (not raw Bass): its compile() step legalizes instructions with
    # multiple sync waits into EventSemaphore instructions; the TRN2 engine
    # instruction encodings only fit one wait each.
    nc = bacc.Bacc("TRN2", target_bir_lowering=False, debug=False,
                   num_devices=N_CORES)
    x_d = nc.dram_tensor("x", [K, H], FP16, kind="ExternalInput").ap()
    gamma_d = nc.dram_tensor("gamma", [1, H], FP16, kind="ExternalInput").ap()
    beta_d = nc.dram_tensor("beta", [1, H], FP16, kind="ExternalInput").ap()
    invc2_d = nc.dram_tensor("invc2", [P, 2 * F], FP32, kind="ExternalInput").ap()
    ustrict_d = nc.dram_tensor("ustrict", [P, P], FP32, kind="ExternalInput").ap()
    row124_d = nc.dram_tensor("row124", [P, P], FP32, kind="ExternalInput").ap()
    out_d = nc.dram_tensor("out", [K, H], FP16, kind="ExternalOutput").ap()

    # [NSEG, P, FS, H]: frame k = s*(P*FS) + p*FS + f
    fs = F // nseg
    x_v = x_d.rearrange("(s p f) h -> s p f h", p=P, f=fs)
    out_v = out_d.rearrange("(s p f) h -> s p f h", p=P, f=fs)

    with tile.TileContext(nc) as tc:
        with (
            tc.tile_pool(name="xbuf", bufs=1) as xpool,
            tc.tile_pool(name="small", bufs=1) as small,
            tc.tile_pool(name="psum", bufs=2, space="PSUM") as psum,
        ):
            X = xpool.tile([P, F, H], FP16)   # X[:, t, :], t = s*FS + f

            G = small.tile([P, H], FP16)
            nc.sync.dma_start(G[:, :], gamma_d.to_broadcast((P, H)))
            Bt = None
            if with_beta:
                Bt = small.tile([P, H], FP16, tag="beta")
                nc.sync.dma_start(Bt[:, :], beta_d.to_broadcast((P, H)))
            invc2 = small.tile([P, 2 * F], FP32, tag="invc2")
            nc.sync.dma_start(invc2[:, :], invc2_d)
            ustrict = small.tile([P, P], FP32, tag="ustrict")
            nc.sync.dma_start(ustrict[:, :], ustrict_d)
            row124 = small.tile([P, P], FP32, tag="row124")
            nc.sync.dma_start(row124[:, :], row124_d)
            eps_t = small.tile([P, 1], FP32, tag="eps")
            nc.vector.memset(eps_t[:, :], EPS)
            # zb must be produced on ACT: the Square+accum activation below
            # carries a same-engine accumulator wait, and its single encodable
            # sync wait must go to the X-chunk DMA.
            zb = small.tile([P, 1], FP32, tag="zb")
            nc.scalar.memzero(zb[:, :])

            S = small.tile([P, 2 * F], FP32, tag="S")    # sums | sumsqs
            C = small.tile([P, 2 * F], FP32, tag="C")    # global cums
            M = small.tile([P, 2 * F], FP32, tag="M")    # [-mean | E[x^2]]
            Msq = small.tile([P, F], FP32, tag="Msq")
            V = small.tile([P, F], FP32, tag="V")
            ISD = small.tile([P, F], FP32, tag="ISD")
            NMB = small.tile([P, F], FP32, tag="NMB")
            carryS = small.tile([P, 2 * nseg], FP32, tag="carryS")
            sqscr = small.tile([P, H], FP16, tag="sqscr")    # ACT square out
            sumscr = small.tile([P, H], FP16, tag="sumscr")  # DVE TS-sum out
            amracc = small.tile([P, 1], FP32, tag="amracc")  # discarded accum

            carryP = psum.tile([P, 2], FP32)
            pe_touch = psum.tile([1, 1], FP32, tag="pe_touch")

            # touchers: one tiny engine-read per DMA so later ops on that
            # engine (whose encodings fit one sync wait, already used by
            # their same-engine chains) never need to also wait on a DMA sem.
            touch = small.tile([1, nseg], FP16, tag="touch")
            touchv = small.tile([1, nseg], FP16, tag="touchv")

            # strided views pairing the sum and sumsq halves: [P, 2, F]
            Cr = C[:, :].rearrange("p (a b) -> p a b", b=F)
            Mr = M[:, :].rearrange("p (a b) -> p a b", b=F)
            Ir = invc2[:, :].rearrange("p (a b) -> p a b", b=F)

            # absorb the ustrict/row124 DMA wait on PE once
            nc.tensor.matmul(pe_touch[0:1, 0:1], row124[0:1, 0:1],
                             ustrict[0:1, 0:1], start=True, stop=True)

            for _rep in range(reps):
              if rep_barrier and _rep > 0:
                  tc.strict_bb_all_engine_barrier()
              for s in range(nseg):
                t0 = s * fs
                # ---- load + per-frame sum/sumsq for this segment ---------
                nc.sync.dma_start(X[:, t0:t0 + fs, :], x_v[s])
                nc.scalar.copy(touch[0:1, s:s + 1], X[0:1, t0, 0:1])
                nc.vector.tensor_scalar(
                    out=touchv[0:1, s:s + 1], in0=X[0:1, t0, 0:1],
                    scalar1=1.0, scalar2=None, op0=mybir.AluOpType.mult)
                for i, f in enumerate(range(t0, t0 + fs)):
                    nc.scalar.activation(
                        out=sqscr[:, :], in_=X[:, f, :],
                        func=mybir.ActivationFunctionType.Square,
                        bias=zb[:, :], scale=1.0,
                        accum_out=S[:, F + f:F + f + 1],
                    )
                    if i < sum_act_frames:
                        # frame-sum on ACT: Identity+accum
                        nc.scalar.activation(
                            out=sumscr[:, :], in_=X[:, f, :],
                            func=mybir.ActivationFunctionType.Identity,
                            bias=zb[:, :], scale=1.0,
                            accum_out=S[:, f:f + 1],
                        )
                    elif sum_mode == "dve_ts":
                        # frame-sum on DVE: copy+accum at 4x packed mode
                        nc.vector.tensor_scalar(
                            out=sumscr[:, :], in0=X[:, f, :],
                            scalar1=1.0, scalar2=0.0,
                            op0=mybir.AluOpType.mult,
                            op1=mybir.AluOpType.add,
                            accum_out=S[:, f:f + 1],
                        )
                if sum_mode == "reduce":
                    # batched DVE reduce over the segment (1x mode)
                    nc.vector.reduce_sum(S[:, t0:t0 + fs],
                                         X[:, t0:t0 + fs, :],
                                         axis=mybir.AxisListType.X)

                # ---- causal scan for this segment ------------------------
                # intra-partition inclusive prefix over the FS frames each
                # partition owns (fp32 recurrence on DVE)
                nc.vector.tensor_tensor_scan(
                    out=C[:, t0:t0 + fs], data0=S[:, t0:t0 + fs],
                    data1=S[:, t0:t0 + fs], initial=0.0,
                    op0=mybir.AluOpType.add, op1=mybir.AluOpType.bypass)
                nc.vector.tensor_tensor_scan(
                    out=C[:, F + t0:F + t0 + fs], data0=S[:, F + t0:F + t0 + fs],
                    data1=S[:, F + t0:F + t0 + fs], initial=0.0,
                    op0=mybir.AluOpType.add, op1=mybir.AluOpType.bypass)
                # cross-partition exclusive carry (+ previous segment's total):
                # carry[p] = sum_{q<p} seg_total[q] + prev_seg_final
                totals = Cr[:, :, t0 + fs - 1]          # [P, 2] strided
                nc.tensor.matmul(carryP[:, 0:2], ustrict[:, :], totals,
                                 start=True, stop=(s == 0))
                if s > 0:
                    prevfinal = Cr[:, :, t0 - 1]        # already global
                    nc.tensor.matmul(carryP[:, 0:2], row124[:, :], prevfinal,
                                     start=False, stop=True)
                cS = carryS[:, 2 * s:2 * s + 2]
                nc.scalar.copy(cS[:, :], carryP[:, :])
                nc.vector.tensor_scalar_add(C[:, t0:t0 + fs],
                                            C[:, t0:t0 + fs], cS[:, 0:1])
                nc.vector.tensor_scalar_add(C[:, F + t0:F + t0 + fs],
                                            C[:, F + t0:F + t0 + fs],
                                            cS[:, 1:2])

                # ---- stats for this segment ------------------------------
                # M = C * invc2: [-mean | E[x^2]] (both halves via 3D AP).
                # C itself must stay intact: the next segment's carry matmul
                # reads this segment's final cum column.
                nc.vector.tensor_mul(Mr[:, :, t0:t0 + fs], Cr[:, :, t0:t0 + fs],
                                     Ir[:, :, t0:t0 + fs])
                nc.vector.tensor_mul(Msq[:, t0:t0 + fs], M[:, t0:t0 + fs],
                                     M[:, t0:t0 + fs])              # mean^2
                nc.vector.tensor_sub(V[:, t0:t0 + fs], M[:, F + t0:F + t0 + fs],
                                     Msq[:, t0:t0 + fs])            # var
                nc.scalar.activation(out=V[:, t0:t0 + fs], in_=V[:, t0:t0 + fs],
                                     func=mybir.ActivationFunctionType.Sqrt,
                                     bias=eps_t[:, :], scale=1.0)
                nc.vector.reciprocal(ISD[:, t0:t0 + fs], V[:, t0:t0 + fs])
                nc.vector.tensor_mul(NMB[:, t0:t0 + fs], M[:, t0:t0 + fs],
                                     ISD[:, t0:t0 + fs])        # -mean*invstd

                # ---- apply + store for this segment ----------------------
                for f in range(t0, t0 + fs):
                    if apply_mode == "amr":
                        # out = (x*invstd + (-mean*invstd)) * gamma, fused,
                        # one rounding to fp16 at the output.
                        nc.vector.affine_mul_reduce(
                            out=X[:, f, :], accum_out=amracc[:, 0:1],
                            in0=X[:, f, :], in1=G[:, :],
                            scale=ISD[:, f:f + 1], bias=NMB[:, f:f + 1])
                    else:
                        nc.vector.tensor_scalar(
                            out=X[:, f, :], in0=X[:, f, :],
                            scalar1=ISD[:, f:f + 1], scalar2=NMB[:, f:f + 1],
                            op0=mybir.AluOpType.mult, op1=mybir.AluOpType.add)
                        nc.vector.tensor_mul(X[:, f, :], X[:, f, :], G[:, :])
                    if Bt is not None:
                        nc.vector.tensor_add(X[:, f, :], X[:, f, :], Bt[:, :])
                nc.gpsimd.dma_start(out_v[s], X[:, t0:t0 + fs, :])

    # Runs Bacc's compile passes (register allocation, EventSemaphore
    # legalization of multi-wait instructions, nop fusion).
    nc.finalize()
    return nc


_NC_CACHE: dict = {}


def kernel(**inputs: np.ndarray) -> np.ndarray:
    x = np.asarray(inputs["inputs"])
    gamma = np.asarray(inputs["gamma"], dtype=np.float32)
    beta = np.asarray(inputs["beta"], dtype=np.float32)
    assert x.shape == (B, K, H), x.shape

    x16 = np.ascontiguousarray(x.astype(np.float16))
    gamma16 = np.ascontiguousarray(gamma.reshape(1, H).astype(np.float16))
    beta16 = np.ascontiguousarray(beta.reshape(1, H).astype(np.float16))

    with_beta = bool(np.any(beta != 0.0))
    key = (with_beta, 1)
    if key not in _NC_CACHE:
        _NC_CACHE[key] = _build_kernel(with_beta, reps=1)
    nc = _NC_CACHE[key]

    invc2, ustrict, row124 = _make_consts()
    in_maps = [
        {
            "x": np.ascontiguousarray(x16[b]),
            "gamma": gamma16,
            "beta": beta16,
            "invc2": invc2,
            "ustrict": ustrict,
            "row124": row124,
        }
        for b in range(B)
    ]
    res = bass_utils.run_bass_kernel_spmd(nc, in_maps, core_ids=list(range(N_CORES)))
    out = np.stack([res.results[b]["out"] for b in range(B)], axis=0)
    return out.astype(np.float32)
